# revision 11
# baseline (speedup 1.0000x reference)
"""Trainium2 Bass kernel for nn_CIN: 3-layer compressed-interaction network.

Reference computation (per layer l with kernel k_l [O,H,M]):
    x_{l+1}[b,o,d] = sum_{h,m} x_l[b,h,d] * x0[b,m,d] * k_l[o,h,m]
    out = concat_l(sum_d x_{l+1}[b,o,d])          # (B, 3*128)

Sharding: pure data-parallel over B across 8 cores (512 batch each).

v2: single-bf16 matmuls (tolerance 2e-2 >> bf16 error ~1e-3), rebalanced
3-engine m-contraction (DVE fused STT + ACT copies + GPSIMD wide
accumulate), out1 via tiny indicator matmul on the PE, phase B in bf16.

Per-core algorithm (bd = 512*16 = 8192 columns, 64 chunks of 128):
  L0/L1: for each bd-chunk c, x_l[:, c] is the PE stationary operand and
  the host-pretransposed kernel k_l_p[h, (m,o)] streams through in bf16;
  PSUM gets t'[col, (m,o)] for 4 m's per matmul.  L0 packs two m-groups
  at partition rows 0-39 / 64-103 (concurrent row-group matmuls).
  The data-dependent m-contraction z[col,o] = sum_m x0'[col,m]*t'_m[col,o]
  splits three ways per group of 4 m's:
    D_G groups: DVE fused scalar_tensor_tensor straight from PSUM
    B_G groups: ACT bulk-copies the group PSUM->SBUF bf16, DVE STT on SBUF
    G_G groups: per-m ACT scaled-copy into a wide tile, GPSIMD 512-wide
                accumulate, folded at finish
  Partial accumulators merge at chunk end (final merge on GPSIMD, written
  directly in bf16).  L0's merged z is PE-transposed to x1 [h, col]
  (bf16); a tiny N=8 indicator matmul on z gives out1 (d-sums) on the PE.
  L1's z merges straight into x2T (bf16) -- what phase B needs.

  L2 (d-sum folded into PE): out3[b,o] = sum_{hm} k2[o,h,m] * w[b,h,m]
  with w[b,h,m] = sum_d x2[b,h,d]*x0[b,m,d].  Per chunk, one bf16 matmul
  lhsT=x2T_c[col,h], rhs=X0E[col,(m,b')] where X0E = x0'[col,m]*E[col,b']
  (E = batch indicator) contracts col, yielding w[h,(m,b')] plus, via an
  appended plain-indicator block, the d-sum of x2 (=out2).  A final
  40-matmul PSUM accumulation k2p_m @ w_m produces out3[o,b].
"""

import numpy as np
from contextlib import ExitStack

import ml_dtypes
import concourse.bass as bass
import concourse.tile as tile
import concourse.mybir as mybir

F32 = mybir.dt.float32
BF16 = mybir.dt.bfloat16
ALU = mybir.AluOpType
AFT = mybir.ActivationFunctionType

B, M, D, O = 4096, 40, 16, 128
N_CORES = 8
BC = B // N_CORES          # 512 batch rows per core
MPG = 4                    # m's packed per matmul (4*128 = 512 free cols)
GROUPS = M // MPG          # 10
M1 = M + 1                 # x0t carries a trailing ones-column

# m-contraction group split per layer (must sum to GROUPS):
D_G = 0                    # DVE direct STT from PSUM
B_G = 6                    # ACT bulk-copy -> SBUF bf16, DVE STT on SBUF
G_G = GROUPS - D_G - B_G   # ACT scaled-copy + GPSIMD wide accumulate

_ns_ctr = [0]


def _split_excess_waits(nc, max_waits=1):
    """walrus in this env rejects >1 sync-wait on one instruction (CTRL
    struct): move excess waits onto same-engine NoOps inserted before."""
    for f in nc.m.functions:
        for bb in f.blocks:
            new_list = []
            for inst in bb.instructions:
                si = inst.sync_info
                waits = list(si.on_wait) if si and si.on_wait else []
                if len(waits) > max_waits:
                    excess = waits[:-max_waits]
                    keep = waits[-max_waits:]
                    for i in range(0, len(excess), max_waits):
                        chunk = excess[i:i + max_waits]
                        _ns_ctr[0] += 1
                        nop = mybir.InstNoOp(
                            name=f"waitsplit-{_ns_ctr[0]}", ins=[], outs=[],
                            engine=inst.engine,
                            sync_info=mybir.SyncInfo(on_wait=chunk, on_update=[]),
                        )
                        nc.register_instruction(nop)
                        new_list.append(nop)
                    si.on_wait = keep
                    inst.sync_info = si
                new_list.append(inst)
            bb.instructions[:] = new_list


def build(n_chunks):
    """Build the per-core Bass program for n_chunks*128 bd columns."""
    bd = n_chunks * 128
    bcl = bd // D              # local batch count
    nb = (bcl + 127) // 128    # output b-tiles
    nc = bass.Bass("TRN2", target_bir_lowering=False, debug=False, num_devices=1)

    x0p2_d = nc.dram_tensor("x0p2", [128, bd], BF16, kind="ExternalInput")
    x0t_d = nc.dram_tensor("x0t", [bd, M1], F32, kind="ExternalInput")
    k0p2_d = nc.dram_tensor("k0p2", [128, M * O // 2], BF16,
                            kind="ExternalInput")
    k1p_d = nc.dram_tensor("k1p", [O, M * O], BF16, kind="ExternalInput")
    k2p_d = nc.dram_tensor("k2p", [O, M * O], BF16, kind="ExternalInput")
    e41_d = nc.dram_tensor("e41", [128, M1 * 8], BF16, kind="ExternalInput")
    e8f_d = nc.dram_tensor("e8f", [128, 8], F32, kind="ExternalInput")
    idenf_d = nc.dram_tensor("idenf", [128, 128], F32, kind="ExternalInput")
    out_d = nc.dram_tensor("out", [bcl, 3 * O], F32, kind="ExternalOutput")

    with tile.TileContext(nc) as tc:
        with ExitStack() as perm:
            pp = perm.enter_context(tc.tile_pool(name="perm", bufs=1))
            x0t_sb = pp.tile([128, n_chunks * M1], F32, name="x0t_sb")
            nc.sync.dma_start(
                x0t_sb[:].rearrange("p (c m) -> p c m", m=M1),
                x0t_d.ap().rearrange("(c p) m -> p c m", p=128))
            x0tb_sb = pp.tile([128, n_chunks * M1], BF16, name="x0tb_sb")
            nc.vector.tensor_copy(x0tb_sb[:], x0t_sb[:])
            e41_sb = pp.tile([128, M1 * 8], BF16, name="e41_sb")
            nc.sync.dma_start(e41_sb[:], e41_d.ap())
            e8f_sb = pp.tile([128, 8], F32, name="e8f_sb")
            nc.sync.dma_start(e8f_sb[:], e8f_d.ap())
            idenf_sb = pp.tile([128, 128], F32, name="idenf_sb")
            nc.sync.dma_start(idenf_sb[:], idenf_d.ap())
            k1p_sb = pp.tile([O, M * O], BF16, name="k1p_sb")
            nc.sync.dma_start(k1p_sb[:], k1p_d.ap())
            x2T_sb = pp.tile([128, bd], BF16, name="x2T_sb")
            o1_st = pp.tile([128, bcl], F32, name="o1_st")
            o2_st = pp.tile([128, bcl], F32, name="o2_st")
            o3_st = pp.tile([128, bcl], F32, name="o3_st")

            def sc_ap(c, m):
                return x0t_sb[:, c * M1 + m: c * M1 + m + 1]

            class StepB:
                """m-contraction consuming t'-groups, split 3 ways."""

                def __init__(self, c, accs_d, acc_g4, fold2, merge_out,
                             tg_pool):
                    self.c, self.accs_d, self.acc_g4 = c, accs_d, acc_g4
                    self.fold2, self.merge_out = fold2, merge_out
                    self.tg_pool = tg_pool
                    self.first_d = [True] * len(accs_d)
                    self.di = 0
                    self.first_g = True

                def _dve(self, src, sc):
                    i = self.di
                    self.di = (i + 1) % len(self.accs_d)
                    if self.first_d[i]:
                        nc.vector.tensor_scalar(
                            self.accs_d[i], src, sc, None, ALU.mult)
                        self.first_d[i] = False
                    else:
                        nc.vector.scalar_tensor_tensor(
                            self.accs_d[i], src, sc, self.accs_d[i],
                            ALU.mult, ALU.add)

                def eat(self, g, pt):
                    c = self.c
                    if g < D_G:
                        for j in range(MPG):
                            self._dve(pt[:, j * O:(j + 1) * O],
                                      sc_ap(c, g * MPG + j))
                    elif g < D_G + B_G:
                        tb = self.tg_pool.tile([128, MPG * O], BF16,
                                               name="tbk", tag="tbk")
                        nc.scalar.copy(tb[:], pt[:])
                        for j in range(MPG):
                            self._dve(tb[:, j * O:(j + 1) * O],
                                      sc_ap(c, g * MPG + j))
                    else:
                        tg = self.tg_pool.tile([128, MPG * O], F32,
                                               name="tg", tag="tg")
                        for j in range(MPG):
                            nc.scalar.activation(
                                tg[:, j * O:(j + 1) * O],
                                pt[:, j * O:(j + 1) * O],
                                AFT.Copy, scale=sc_ap(c, g * MPG + j))
                        if self.first_g:
                            nc.gpsimd.tensor_copy(self.acc_g4, tg[:])
                            self.first_g = False
                        else:
                            nc.gpsimd.tensor_tensor(
                                self.acc_g4, self.acc_g4, tg[:], ALU.add)

                def finish(self):
                    # fold acc_g4 [128,512] -> [128,128] on GPS (all fp32),
                    # final dtype-converting merge on DVE
                    nc.gpsimd.tensor_tensor(
                        self.fold2,
                        self.acc_g4[:, 0:2 * O], self.acc_g4[:, 2 * O:4 * O],
                        ALU.add)
                    nc.vector.tensor_tensor(
                        self.accs_d[0], self.accs_d[0], self.accs_d[1],
                        ALU.add)
                    nc.gpsimd.tensor_tensor(
                        self.fold2[:, 0:O],
                        self.fold2[:, 0:O], self.fold2[:, O:2 * O], ALU.add)
                    nc.vector.tensor_tensor(
                        self.merge_out, self.accs_d[0], self.fold2[:, 0:O],
                        ALU.add)

            with ExitStack() as phA:
                pa = phA.enter_context(tc.tile_pool(name="phA", bufs=1))
                x0p2_sb = pa.tile([128, bd], BF16, name="x0p2_sb")
                nc.sync.dma_start(x0p2_sb[:], x0p2_d.ap())
                k0p2_sb = pa.tile([128, M * O // 2], BF16, name="k0p2_sb")
                nc.sync.dma_start(k0p2_sb[:], k0p2_d.ap())
                x1_sb = pa.tile([128, bd], BF16, name="x1_sb")
                acc_pool = phA.enter_context(tc.tile_pool(name="accs", bufs=4))
                accg_pool = phA.enter_context(tc.tile_pool(name="accg", bufs=2))
                tg_pool = phA.enter_context(tc.tile_pool(name="tgs", bufs=6))
                pt_pool = phA.enter_context(
                    tc.tile_pool(name="ptp", bufs=4, space="PSUM"))
                ptr_pool = phA.enter_context(
                    tc.tile_pool(name="ptrp", bufs=2, space="PSUM"))
                po1_pool = phA.enter_context(
                    tc.tile_pool(name="po1p", bufs=1, space="PSUM"))
                po1 = po1_pool.tile([128, bcl], F32, name="po1")

                def stepb_make(c, merge_out):
                    acc_d = acc_pool.tile([128, 128], F32, name="acc_d",
                                          tag="acc_d")
                    acc_d2 = acc_pool.tile([128, 128], F32, name="acc_d2",
                                           tag="acc_d2")
                    acc_g4 = accg_pool.tile([128, MPG * O], F32,
                                            name="acc_g4", tag="acc_g4")
                    fold2 = acc_pool.tile([128, 2 * O], F32, name="fold2",
                                          tag="fold2")
                    return StepB(c, [acc_d[:], acc_d2[:]], acc_g4[:],
                                 fold2[:], merge_out, tg_pool)

                # L0 (row-packed 2x: strips at partitions 0-39 and 64-103)
                # -> merge -> out1 mini-MM + PE transpose -> x1 (bf16)
                for c in range(n_chunks):
                    cs = slice(c * 128, (c + 1) * 128)
                    accm = acc_pool.tile([128, 128], F32, name="accm",
                                         tag="accm")
                    sb = stepb_make(c, accm[:])
                    for p in range(GROUPS // 2):
                        pt_a = pt_pool.tile([128, MPG * O], F32, name="pt",
                                            tag="pt")
                        pt_b = pt_pool.tile([128, MPG * O], F32, name="pt2",
                                            tag="pt")
                        ps = slice(p * MPG * O, (p + 1) * MPG * O)
                        nc.tensor.matmul(pt_a[:], x0p2_sb[0:M, cs],
                                         k0p2_sb[0:M, ps],
                                         start=True, stop=True)
                        nc.tensor.matmul(pt_b[:], x0p2_sb[64:64 + M, cs],
                                         k0p2_sb[64:64 + M, ps],
                                         start=True, stop=True)
                        sb.eat(2 * p, pt_a)
                        sb.eat(2 * p + 1, pt_b)
                    sb.finish()
                    # out1 contribution: po1[:, 8c:8c+8] = accm^T @ e8
                    nc.tensor.matmul(po1[:, c * 8:(c + 1) * 8], accm[:],
                                     e8f_sb[:], start=True, stop=True)
                    ptr = ptr_pool.tile([128, 128], F32, name="ptr",
                                        tag="ptr")
                    nc.tensor.transpose(ptr[:], accm[:], idenf_sb[:])
                    nc.scalar.copy(x1_sb[:, cs], ptr[:])
                nc.scalar.copy(o1_st[:], po1[:])

                # L1: merge straight into resident x2T slices (bf16)
                for c in range(n_chunks):
                    cs = slice(c * 128, (c + 1) * 128)
                    sb = stepb_make(c, x2T_sb[:, cs])
                    for g in range(GROUPS):
                        pt = pt_pool.tile([128, MPG * O], F32, name="pt",
                                          tag="pt")
                        gs = slice(g * MPG * O, (g + 1) * MPG * O)
                        nc.tensor.matmul(pt[:], x1_sb[:, cs], k1p_sb[:, gs],
                                         start=True, stop=True)
                        sb.eat(g, pt)
                    sb.finish()

            # ---- phase B: L2 via indicator matmuls (bf16) ----
            with ExitStack() as phB:
                pb = phB.enter_context(tc.tile_pool(name="phB", bufs=1))
                k2p_sb = pb.tile([O, M * O], BF16, name="k2p_sb")
                nc.sync.dma_start(k2p_sb[:], k2p_d.ap())
                w_sb = pb.tile([128, M * bcl], BF16, name="w_sb")
                x0e_pool = phB.enter_context(tc.tile_pool(name="x0es", bufs=3))
                pw_pool = phB.enter_context(
                    tc.tile_pool(name="pwp", bufs=3, space="PSUM"))
                po3_pool = phB.enter_context(
                    tc.tile_pool(name="po3p", bufs=1, space="PSUM"))
                ptp_pool = phB.enter_context(
                    tc.tile_pool(name="ptpp", bufs=2, space="PSUM"))

                e41_3d = e41_sb[:].rearrange("p (m e) -> p m e", e=8)
                w_4d = w_sb[:].rearrange("p (m b) -> p m b", b=bcl)
                for c in range(n_chunks):
                    x0e = x0e_pool.tile([128, M1 * 8], BF16, name="x0e",
                                        tag="x0e")
                    nc.vector.tensor_tensor(
                        x0e[:].rearrange("p (m e) -> p m e", e=8),
                        x0tb_sb[:, c * M1:(c + 1) * M1].unsqueeze(2)
                        .broadcast_to([128, M1, 8]),
                        e41_3d, ALU.mult)
                    pw = pw_pool.tile([128, M1 * 8], F32, name="pw", tag="pw")
                    nc.tensor.matmul(
                        pw[:], x2T_sb[:, c * 128:(c + 1) * 128],
                        x0e[:], start=True, stop=True)
                    # scatter w part: w[h, m, c*8+j] <- pw[h, m*8+j]
                    nc.scalar.copy(
                        w_4d[:, :, c * 8:(c + 1) * 8],
                        pw[:, 0:M * 8].rearrange("p (m e) -> p m e", e=8))
                    # out2 block: columns [320:328)
                    nc.scalar.copy(o2_st[:, c * 8:(c + 1) * 8],
                                   pw[:, M * 8:M1 * 8])

                po3 = po3_pool.tile([128, bcl], F32, name="po3")
                for m in range(M):
                    nc.tensor.matmul(
                        po3[:], k2p_sb[:, m * O:(m + 1) * O],
                        w_sb[:, m * bcl:(m + 1) * bcl],
                        start=(m == 0), stop=(m == M - 1))
                nc.scalar.copy(o3_st[:], po3[:])

                # ---- outputs: transpose [o, b] tiles to [b, o] and store
                tb_pool = phB.enter_context(tc.tile_pool(name="tbs", bufs=3))
                for l, st in enumerate((o1_st, o2_st, o3_st)):
                    for j in range(nb):
                        tw = min(128, bcl - j * 128)
                        ptp = ptp_pool.tile([128, 128], F32, name="ptp",
                                            tag="ptp")
                        nc.tensor.transpose(
                            ptp[0:tw, :], st[:, j * 128:j * 128 + tw],
                            idenf_sb[:])
                        tb = tb_pool.tile([128, 128], F32, name="tb", tag="tb")
                        nc.scalar.copy(tb[0:tw, :], ptp[0:tw, :])
                        nc.sync.dma_start(
                            out_d.ap()[j * 128:j * 128 + tw,
                                       l * O:(l + 1) * O],
                            tb[0:tw, :])

    _split_excess_waits(nc)
    return nc


def host_prep(x0c, k0, k1, k2):
    """Per-core input prep. x0c: (bcl, M, D) float32."""
    bcl = x0c.shape[0]
    x0m = np.ascontiguousarray(
        x0c.transpose(1, 0, 2).reshape(M, bcl * D), dtype=np.float32)
    x0mb = x0m.astype(ml_dtypes.bfloat16)
    x0p2 = np.zeros((128, bcl * D), ml_dtypes.bfloat16)
    x0p2[0:M] = x0mb
    x0p2[64:64 + M] = x0mb
    x0t = np.concatenate(
        [x0c.transpose(0, 2, 1).reshape(bcl * D, M),
         np.ones((bcl * D, 1), np.float32)], axis=1)
    x0t = np.ascontiguousarray(x0t, dtype=np.float32)
    k0p = np.ascontiguousarray(
        k0.transpose(1, 2, 0).reshape(M, M * O), dtype=np.float32)
    k0pb = k0p.astype(ml_dtypes.bfloat16)
    npair = GROUPS // 2
    k0p2 = np.zeros((128, M * O // 2), ml_dtypes.bfloat16)
    for p in range(npair):
        ps = slice(p * MPG * O, (p + 1) * MPG * O)
        k0p2[0:M, ps] = k0pb[:, (2 * p) * MPG * O:(2 * p + 1) * MPG * O]
        k0p2[64:64 + M, ps] = k0pb[:, (2 * p + 1) * MPG * O:
                                   (2 * p + 2) * MPG * O]
    k1p = np.ascontiguousarray(
        k1.transpose(1, 2, 0).reshape(O, M * O)).astype(ml_dtypes.bfloat16)
    k2p = np.ascontiguousarray(
        k2.transpose(1, 2, 0).reshape(O, M * O)).astype(ml_dtypes.bfloat16)
    e8 = (np.arange(128)[:, None] // D == np.arange(8)[None, :])
    e8b = e8.astype(ml_dtypes.bfloat16)
    e41 = np.ascontiguousarray(
        np.tile(e8b[:, None, :], (1, M1, 1)).reshape(128, M1 * 8))
    idenf = np.eye(128, dtype=np.float32)
    return {"x0p2": x0p2, "x0t": x0t, "k0p2": k0p2, "k1p": k1p,
            "k2p": k2p, "e41": e41, "e8f": e8.astype(np.float32),
            "idenf": idenf}


_nc_cache = {}


def _get_nc(n_chunks):
    if n_chunks not in _nc_cache:
        _nc_cache[n_chunks] = build(n_chunks)
    return _nc_cache[n_chunks]


def kernel(x0, k0, k1, k2):
    from concourse.bass_utils import run_bass_kernel_spmd
    x0 = np.asarray(x0, dtype=np.float32)
    k0 = np.asarray(k0, dtype=np.float32)
    k1 = np.asarray(k1, dtype=np.float32)
    k2 = np.asarray(k2, dtype=np.float32)
    n_chunks = (BC * D) // 128
    nc = _get_nc(n_chunks)
    in_maps = [host_prep(x0[c * BC:(c + 1) * BC], k0, k1, k2)
               for c in range(N_CORES)]
    res = run_bass_kernel_spmd(nc, in_maps, core_ids=list(range(N_CORES)))
    out = np.concatenate([r["out"] for r in res.results], axis=0)
    return out.astype(np.float32)


# revision 13
# speedup vs baseline: 1.4650x; 1.4650x over previous
"""Trainium2 Bass kernel for nn_CIN: 3-layer compressed-interaction network.

Reference computation (per layer l with kernel k_l [O,H,M]):
    x_{l+1}[b,o,d] = sum_{h,m} x_l[b,h,d] * x0[b,m,d] * k_l[o,h,m]
    out = concat_l(sum_d x_{l+1}[b,o,d])          # (B, 3*128)

Sharding: pure data-parallel over B across 8 cores (512 batch each).

v3 ("diag-matmul"): the data-dependent m-contraction
    z[col,o] = sum_m x0'[col,m] * t'_m[col,o]
runs entirely on the TensorEngine as 40 PSUM-accumulated matmuls per
chunk-layer against materialized diagonal matrices diag(x0'[:,m]).
All 40 bf16 diagonals for a chunk are built in ONE wide DVE op
(x0-broadcast times an identity mask) and shared by L0 and L1 (the
chunk loops are fused).  Two matmul orientations avoid all transposes:
  L0: lhsT = t'_m [col,h-block] (stationary), rhs = diag_m
      -> out zT[h, col] accumulated over m = x1 orientation directly.
  L1: lhsT = diag_m (stationary), rhs = t'_m [col,o]
      -> out z[col, o] = x2T orientation directly.
t' tiles move PSUM->SBUF (bf16 downcast) via ACT/DVE copies split
between the two engines.  out1 = one strided DVE reduce over x1.
Everything matmul is single bf16 (tolerance 2e-2 >> bf16 error ~4e-3).

  L2 (d-sum folded into PE): out3[b,o] = sum_{hm} k2[o,h,m] * w[b,h,m]
  with w[b,h,m] = sum_d x2[b,h,d]*x0[b,m,d].  Per chunk, one bf16 matmul
  lhsT=x2T_c[col,h], rhs=X0E[col,(m,b')] where X0E = x0'[col,m]*E[col,b']
  (E = batch indicator) contracts col, yielding w[h,(m,b')] plus, via an
  appended plain-indicator block, the d-sum of x2 (=out2).  A final
  40-matmul PSUM accumulation k2p_m @ w_m produces out3[o,b].
"""

import numpy as np
from contextlib import ExitStack

import ml_dtypes
import concourse.bass as bass
import concourse.tile as tile
import concourse.mybir as mybir

F32 = mybir.dt.float32
BF16 = mybir.dt.bfloat16
ALU = mybir.AluOpType
AFT = mybir.ActivationFunctionType

B, M, D, O = 4096, 40, 16, 128
N_CORES = 8
BC = B // N_CORES          # 512 batch rows per core
MPG = 4                    # m's packed per matmul (4*128 = 512 free cols)
GROUPS = M // MPG          # 10
M1 = M + 1                 # x0t carries a trailing ones-column

_ns_ctr = [0]


def _split_excess_waits(nc, max_waits=1):
    """walrus in this env rejects >1 sync-wait on one instruction (CTRL
    struct): move excess waits onto same-engine NoOps inserted before."""
    for f in nc.m.functions:
        for bb in f.blocks:
            new_list = []
            for inst in bb.instructions:
                si = inst.sync_info
                waits = list(si.on_wait) if si and si.on_wait else []
                if len(waits) > max_waits:
                    excess = waits[:-max_waits]
                    keep = waits[-max_waits:]
                    for i in range(0, len(excess), max_waits):
                        chunk = excess[i:i + max_waits]
                        _ns_ctr[0] += 1
                        nop = mybir.InstNoOp(
                            name=f"waitsplit-{_ns_ctr[0]}", ins=[], outs=[],
                            engine=inst.engine,
                            sync_info=mybir.SyncInfo(on_wait=chunk, on_update=[]),
                        )
                        nc.register_instruction(nop)
                        new_list.append(nop)
                    si.on_wait = keep
                    inst.sync_info = si
                new_list.append(inst)
            bb.instructions[:] = new_list


def build(n_chunks):
    """Build the per-core Bass program for n_chunks*128 bd columns."""
    bd = n_chunks * 128
    bcl = bd // D              # local batch count
    nb = (bcl + 127) // 128    # output b-tiles
    nc = bass.Bass("TRN2", target_bir_lowering=False, debug=False, num_devices=1)

    x0p2_d = nc.dram_tensor("x0p2", [128, bd], BF16, kind="ExternalInput")
    x0t_d = nc.dram_tensor("x0t", [bd, M1], F32, kind="ExternalInput")
    k0p2_d = nc.dram_tensor("k0p2", [128, M * O // 2], BF16,
                            kind="ExternalInput")
    k1p_d = nc.dram_tensor("k1p", [O, M * O], BF16, kind="ExternalInput")
    k2p_d = nc.dram_tensor("k2p", [O, M * O], BF16, kind="ExternalInput")
    e41_d = nc.dram_tensor("e41", [128, M1 * 8], BF16, kind="ExternalInput")
    idenb_d = nc.dram_tensor("idenb", [128, 128], BF16, kind="ExternalInput")
    idenf_d = nc.dram_tensor("idenf", [128, 128], F32, kind="ExternalInput")
    out_d = nc.dram_tensor("out", [bcl, 3 * O], F32, kind="ExternalOutput")

    with tile.TileContext(nc) as tc:
        with ExitStack() as perm:
            pp = perm.enter_context(tc.tile_pool(name="perm", bufs=1))
            x0t_sb = pp.tile([128, n_chunks * M1], F32, name="x0t_sb")
            nc.sync.dma_start(
                x0t_sb[:].rearrange("p (c m) -> p c m", m=M1),
                x0t_d.ap().rearrange("(c p) m -> p c m", p=128))
            x0tb_sb = pp.tile([128, n_chunks * M1], BF16, name="x0tb_sb")
            nc.vector.tensor_copy(x0tb_sb[:], x0t_sb[:])
            e41_sb = pp.tile([128, M1 * 8], BF16, name="e41_sb")
            nc.sync.dma_start(e41_sb[:], e41_d.ap())
            idenb_sb = pp.tile([128, 128], BF16, name="idenb_sb")
            nc.sync.dma_start(idenb_sb[:], idenb_d.ap())
            idenf_sb = pp.tile([128, 128], F32, name="idenf_sb")
            nc.sync.dma_start(idenf_sb[:], idenf_d.ap())
            k1p_sb = pp.tile([O, M * O], BF16, name="k1p_sb")
            nc.sync.dma_start(k1p_sb[:], k1p_d.ap())
            x2T_sb = pp.tile([128, bd], BF16, name="x2T_sb")
            o1_st = pp.tile([128, bcl], F32, name="o1_st")
            o2_st = pp.tile([128, bcl], F32, name="o2_st")
            o3_st = pp.tile([128, bcl], F32, name="o3_st")

            with ExitStack() as phA:
                pa = phA.enter_context(tc.tile_pool(name="phA", bufs=1))
                x0p2_sb = pa.tile([128, bd], BF16, name="x0p2_sb")
                nc.sync.dma_start(x0p2_sb[:], x0p2_d.ap())
                k0p2_sb = pa.tile([128, M * O // 2], BF16, name="k0p2_sb")
                nc.sync.dma_start(k0p2_sb[:], k0p2_d.ap())
                x1_sb = pa.tile([128, bd], BF16, name="x1_sb")
                dg_pool = phA.enter_context(tc.tile_pool(name="dgs", bufs=2))
                ts_pool = phA.enter_context(tc.tile_pool(name="tss", bufs=12))
                pt_pool = phA.enter_context(
                    tc.tile_pool(name="ptp", bufs=4, space="PSUM"))
                z_pool = phA.enter_context(
                    tc.tile_pool(name="zp", bufs=4, space="PSUM"))

                for c in range(n_chunks):
                    cs = slice(c * 128, (c + 1) * 128)
                    # all 40 diag(x0'[:,m]) for this chunk in one DVE op
                    dg = dg_pool.tile([128, M * 128], BF16, name="dg",
                                      tag="dg")
                    nc.vector.tensor_tensor(
                        dg[:].rearrange("p (m q) -> p m q", q=128),
                        x0tb_sb[:, c * M1:c * M1 + M].unsqueeze(2)
                        .broadcast_to([128, M, 128]),
                        idenb_sb[:].unsqueeze(1).broadcast_to([128, M, 128]),
                        ALU.mult)

                    def dgm(m):
                        return dg[:, m * 128:(m + 1) * 128]

                    # ---- L0: h-contraction (row-packed pairs) + copies
                    tsbs = []
                    for p in range(GROUPS // 2):
                        pt_a = pt_pool.tile([128, MPG * O], F32, name="pt",
                                            tag="pt")
                        pt_b = pt_pool.tile([128, MPG * O], F32, name="pt2",
                                            tag="pt")
                        ps = slice(p * MPG * O, (p + 1) * MPG * O)
                        nc.tensor.matmul(pt_a[:], x0p2_sb[0:M, cs],
                                         k0p2_sb[0:M, ps],
                                         start=True, stop=True)
                        nc.tensor.matmul(pt_b[:], x0p2_sb[64:64 + M, cs],
                                         k0p2_sb[64:64 + M, ps],
                                         start=True, stop=True)
                        ta = ts_pool.tile([128, MPG * O], BF16, name="ts",
                                          tag="ts")
                        tb = ts_pool.tile([128, MPG * O], BF16, name="ts2",
                                          tag="ts")
                        if p % 2 == 0:
                            nc.scalar.copy(ta[:], pt_a[:])
                            nc.vector.tensor_copy(tb[:], pt_b[:])
                        else:
                            nc.vector.tensor_copy(ta[:], pt_a[:])
                            nc.scalar.copy(tb[:], pt_b[:])
                        tsbs.append(ta)
                        tsbs.append(tb)
                    # ---- L0 m-contraction: zT[h,col] += t'_m^T scaled
                    z0 = z_pool.tile([128, 128], F32, name="z0", tag="z")
                    for m in range(M):
                        g, j = divmod(m, MPG)
                        nc.tensor.matmul(
                            z0[:], tsbs[g][:, j * O:(j + 1) * O], dgm(m),
                            start=(m == 0), stop=(m == M - 1))
                    nc.scalar.copy(x1_sb[:, cs], z0[:])

                    # ---- L1: h-contraction + copies
                    tsbs1 = []
                    for g2 in range(GROUPS // 2):
                        pt_a = pt_pool.tile([128, MPG * O], F32, name="pt",
                                            tag="pt")
                        pt_b = pt_pool.tile([128, MPG * O], F32, name="pt2",
                                            tag="pt")
                        gs_a = slice((2 * g2) * MPG * O,
                                     (2 * g2 + 1) * MPG * O)
                        gs_b = slice((2 * g2 + 1) * MPG * O,
                                     (2 * g2 + 2) * MPG * O)
                        nc.tensor.matmul(pt_a[:], x1_sb[:, cs],
                                         k1p_sb[:, gs_a],
                                         start=True, stop=True)
                        nc.tensor.matmul(pt_b[:], x1_sb[:, cs],
                                         k1p_sb[:, gs_b],
                                         start=True, stop=True)
                        ta = ts_pool.tile([128, MPG * O], BF16, name="ts",
                                          tag="ts")
                        tb = ts_pool.tile([128, MPG * O], BF16, name="ts2",
                                          tag="ts")
                        if g2 % 2 == 0:
                            nc.scalar.copy(ta[:], pt_a[:])
                            nc.vector.tensor_copy(tb[:], pt_b[:])
                        else:
                            nc.vector.tensor_copy(ta[:], pt_a[:])
                            nc.scalar.copy(tb[:], pt_b[:])
                        tsbs1.append(ta)
                        tsbs1.append(tb)
                    # ---- L1 m-contraction: z[col,o] += diag_m @ t'_m
                    z1 = z_pool.tile([128, 128], F32, name="z1", tag="z")
                    for m in range(M):
                        g, j = divmod(m, MPG)
                        nc.tensor.matmul(
                            z1[:], dgm(m), tsbs1[g][:, j * O:(j + 1) * O],
                            start=(m == 0), stop=(m == M - 1))
                    nc.scalar.copy(x2T_sb[:, cs], z1[:])

                # out1 = sum_d x1 (strided reduce, one op)
                nc.vector.tensor_reduce(
                    o1_st[:], x1_sb[:].rearrange("p (b d) -> p b d", d=D),
                    mybir.AxisListType.X, ALU.add)

            # ---- phase B: L2 via indicator matmuls (bf16) ----
            with ExitStack() as phB:
                pb = phB.enter_context(tc.tile_pool(name="phB", bufs=1))
                k2p_sb = pb.tile([O, M * O], BF16, name="k2p_sb")
                nc.sync.dma_start(k2p_sb[:], k2p_d.ap())
                w_sb = pb.tile([128, M * bcl], BF16, name="w_sb")
                x0e_pool = phB.enter_context(tc.tile_pool(name="x0es", bufs=3))
                pw_pool = phB.enter_context(
                    tc.tile_pool(name="pwp", bufs=3, space="PSUM"))
                po3_pool = phB.enter_context(
                    tc.tile_pool(name="po3p", bufs=1, space="PSUM"))
                ptp_pool = phB.enter_context(
                    tc.tile_pool(name="ptpp", bufs=2, space="PSUM"))

                e41_3d = e41_sb[:].rearrange("p (m e) -> p m e", e=8)
                w_4d = w_sb[:].rearrange("p (m b) -> p m b", b=bcl)
                for c in range(n_chunks):
                    x0e = x0e_pool.tile([128, M1 * 8], BF16, name="x0e",
                                        tag="x0e")
                    nc.vector.tensor_tensor(
                        x0e[:].rearrange("p (m e) -> p m e", e=8),
                        x0tb_sb[:, c * M1:(c + 1) * M1].unsqueeze(2)
                        .broadcast_to([128, M1, 8]),
                        e41_3d, ALU.mult)
                    pw = pw_pool.tile([128, M1 * 8], F32, name="pw", tag="pw")
                    nc.tensor.matmul(
                        pw[:], x2T_sb[:, c * 128:(c + 1) * 128],
                        x0e[:], start=True, stop=True)
                    # scatter w part: w[h, m, c*8+j] <- pw[h, m*8+j]
                    nc.scalar.copy(
                        w_4d[:, :, c * 8:(c + 1) * 8],
                        pw[:, 0:M * 8].rearrange("p (m e) -> p m e", e=8))
                    # out2 block: columns [320:328)
                    nc.scalar.copy(o2_st[:, c * 8:(c + 1) * 8],
                                   pw[:, M * 8:M1 * 8])

                po3 = po3_pool.tile([128, bcl], F32, name="po3")
                for m in range(M):
                    nc.tensor.matmul(
                        po3[:], k2p_sb[:, m * O:(m + 1) * O],
                        w_sb[:, m * bcl:(m + 1) * bcl],
                        start=(m == 0), stop=(m == M - 1))
                nc.scalar.copy(o3_st[:], po3[:])

                # ---- outputs: transpose [o, b] tiles to [b, o] and store
                tb_pool = phB.enter_context(tc.tile_pool(name="tbs", bufs=3))
                for l, st in enumerate((o1_st, o2_st, o3_st)):
                    for j in range(nb):
                        tw = min(128, bcl - j * 128)
                        ptp = ptp_pool.tile([128, 128], F32, name="ptp",
                                            tag="ptp")
                        nc.tensor.transpose(
                            ptp[0:tw, :], st[:, j * 128:j * 128 + tw],
                            idenf_sb[:])
                        tb = tb_pool.tile([128, 128], F32, name="tb", tag="tb")
                        nc.scalar.copy(tb[0:tw, :], ptp[0:tw, :])
                        nc.sync.dma_start(
                            out_d.ap()[j * 128:j * 128 + tw,
                                       l * O:(l + 1) * O],
                            tb[0:tw, :])

    _split_excess_waits(nc)
    return nc


def host_prep(x0c, k0, k1, k2):
    """Per-core input prep. x0c: (bcl, M, D) float32."""
    bcl = x0c.shape[0]
    x0m = np.ascontiguousarray(
        x0c.transpose(1, 0, 2).reshape(M, bcl * D), dtype=np.float32)
    x0mb = x0m.astype(ml_dtypes.bfloat16)
    x0p2 = np.zeros((128, bcl * D), ml_dtypes.bfloat16)
    x0p2[0:M] = x0mb
    x0p2[64:64 + M] = x0mb
    x0t = np.concatenate(
        [x0c.transpose(0, 2, 1).reshape(bcl * D, M),
         np.ones((bcl * D, 1), np.float32)], axis=1)
    x0t = np.ascontiguousarray(x0t, dtype=np.float32)
    k0p = np.ascontiguousarray(
        k0.transpose(1, 2, 0).reshape(M, M * O), dtype=np.float32)
    k0pb = k0p.astype(ml_dtypes.bfloat16)
    npair = GROUPS // 2
    k0p2 = np.zeros((128, M * O // 2), ml_dtypes.bfloat16)
    for p in range(npair):
        ps = slice(p * MPG * O, (p + 1) * MPG * O)
        k0p2[0:M, ps] = k0pb[:, (2 * p) * MPG * O:(2 * p + 1) * MPG * O]
        k0p2[64:64 + M, ps] = k0pb[:, (2 * p + 1) * MPG * O:
                                   (2 * p + 2) * MPG * O]
    k1p = np.ascontiguousarray(
        k1.transpose(1, 2, 0).reshape(O, M * O)).astype(ml_dtypes.bfloat16)
    k2p = np.ascontiguousarray(
        k2.transpose(1, 2, 0).reshape(O, M * O)).astype(ml_dtypes.bfloat16)
    e8 = (np.arange(128)[:, None] // D == np.arange(8)[None, :])
    e8b = e8.astype(ml_dtypes.bfloat16)
    e41 = np.ascontiguousarray(
        np.tile(e8b[:, None, :], (1, M1, 1)).reshape(128, M1 * 8))
    idenb = np.eye(128, dtype=ml_dtypes.bfloat16)
    idenf = np.eye(128, dtype=np.float32)
    return {"x0p2": x0p2, "x0t": x0t, "k0p2": k0p2, "k1p": k1p,
            "k2p": k2p, "e41": e41, "idenb": idenb, "idenf": idenf}


_nc_cache = {}


def _get_nc(n_chunks):
    if n_chunks not in _nc_cache:
        _nc_cache[n_chunks] = build(n_chunks)
    return _nc_cache[n_chunks]


def kernel(x0, k0, k1, k2):
    from concourse.bass_utils import run_bass_kernel_spmd
    x0 = np.asarray(x0, dtype=np.float32)
    k0 = np.asarray(k0, dtype=np.float32)
    k1 = np.asarray(k1, dtype=np.float32)
    k2 = np.asarray(k2, dtype=np.float32)
    n_chunks = (BC * D) // 128
    nc = _get_nc(n_chunks)
    in_maps = [host_prep(x0[c * BC:(c + 1) * BC], k0, k1, k2)
               for c in range(N_CORES)]
    res = run_bass_kernel_spmd(nc, in_maps, core_ids=list(range(N_CORES)))
    out = np.concatenate([r["out"] for r in res.results], axis=0)
    return out.astype(np.float32)


# revision 17
# speedup vs baseline: 1.9271x; 1.3154x over previous
"""Trainium2 Bass kernel for nn_CIN: 3-layer compressed-interaction network.

Reference computation (per layer l with kernel k_l [O,H,M]):
    x_{l+1}[b,o,d] = sum_{h,m} x_l[b,h,d] * x0[b,m,d] * k_l[o,h,m]
    out = concat_l(sum_d x_{l+1}[b,o,d])          # (B, 3*128)

Sharding: pure data-parallel over B across 8 cores (512 batch each).

v3 ("diag-matmul"): the data-dependent m-contraction
    z[col,o] = sum_m x0'[col,m] * t'_m[col,o]
runs entirely on the TensorEngine as 40 PSUM-accumulated matmuls per
chunk-layer against materialized diagonal matrices diag(x0'[:,m]).
All 40 bf16 diagonals for a chunk are built in ONE wide DVE op
(x0-broadcast times an identity mask) and shared by L0 and L1 (the
chunk loops are fused).  Two matmul orientations avoid all transposes:
  L0: lhsT = t'_m [col,h-block] (stationary), rhs = diag_m
      -> out zT[h, col] accumulated over m = x1 orientation directly.
  L1: lhsT = diag_m (stationary), rhs = t'_m [col,o]
      -> out z[col, o] = x2T orientation directly.
t' tiles move PSUM->SBUF (bf16 downcast) via ACT/DVE copies split
between the two engines.  out1 = one strided DVE reduce over x1.
Everything matmul is single bf16 (tolerance 2e-2 >> bf16 error ~4e-3).

  L2 (d-sum folded into PE): out3[b,o] = sum_{hm} k2[o,h,m] * w[b,h,m]
  with w[b,h,m] = sum_d x2[b,h,d]*x0[b,m,d].  Per chunk, one bf16 matmul
  lhsT=x2T_c[col,h], rhs=X0E[col,(m,b')] where X0E = x0'[col,m]*E[col,b']
  (E = batch indicator) contracts col, yielding w[h,(m,b')] plus, via an
  appended plain-indicator block, the d-sum of x2 (=out2).  A final
  40-matmul PSUM accumulation k2p_m @ w_m produces out3[o,b].
"""

import os
import numpy as np
from contextlib import ExitStack

import ml_dtypes
import concourse.bass as bass
import concourse.tile as tile
import concourse.mybir as mybir

# The stock compile pipeline passes --enable-ldw-opt=false to walrus.
# Flipping it crashes walrus codegen (visitInstLdweights) in this
# environment, so leave it off unless CIN_LDW_OPT=1 is set explicitly.
if os.environ.get("CIN_LDW_OPT", "0") == "1":
    import concourse.bass_utils as _bu

    if not getattr(_bu, "_cin_ldw_patched", False):
        _orig_run_command = _bu.run_command

        def _run_command_ldw(cmd, *args, **kwargs):
            if isinstance(cmd, (list, tuple)):
                cmd = [c.replace("--enable-ldw-opt=false",
                                 "--enable-ldw-opt=true")
                       if isinstance(c, str) else c for c in cmd]
            return _orig_run_command(cmd, *args, **kwargs)

        _bu.run_command = _run_command_ldw
        _bu._cin_ldw_patched = True

F32 = mybir.dt.float32
BF16 = mybir.dt.bfloat16
ALU = mybir.AluOpType
AFT = mybir.ActivationFunctionType

B, M, D, O = 4096, 40, 16, 128
N_CORES = 8
BC = B // N_CORES          # 512 batch rows per core
MPG = 4                    # m's packed per matmul (4*128 = 512 free cols)
GROUPS = M // MPG          # 10
M1 = M + 1                 # x0t carries a trailing ones-column

_ns_ctr = [0]


def _split_excess_waits(nc, max_waits=1):
    """walrus in this env rejects >1 sync-wait on one instruction (CTRL
    struct): move excess waits onto same-engine NoOps inserted before."""
    for f in nc.m.functions:
        for bb in f.blocks:
            new_list = []
            for inst in bb.instructions:
                si = inst.sync_info
                waits = list(si.on_wait) if si and si.on_wait else []
                if len(waits) > max_waits:
                    excess = waits[:-max_waits]
                    keep = waits[-max_waits:]
                    for i in range(0, len(excess), max_waits):
                        chunk = excess[i:i + max_waits]
                        _ns_ctr[0] += 1
                        nop = mybir.InstNoOp(
                            name=f"waitsplit-{_ns_ctr[0]}", ins=[], outs=[],
                            engine=inst.engine,
                            sync_info=mybir.SyncInfo(on_wait=chunk, on_update=[]),
                        )
                        nc.register_instruction(nop)
                        new_list.append(nop)
                    si.on_wait = keep
                    inst.sync_info = si
                new_list.append(inst)
            bb.instructions[:] = new_list


def build(n_chunks):
    """Build the per-core Bass program for n_chunks*128 bd columns."""
    bd = n_chunks * 128
    bcl = bd // D              # local batch count
    nb = (bcl + 127) // 128    # output b-tiles
    nc = bass.Bass("TRN2", target_bir_lowering=False, debug=False, num_devices=1)

    x0p2_d = nc.dram_tensor("x0p2", [128, bd], BF16, kind="ExternalInput")
    x0t_d = nc.dram_tensor("x0t", [bd, M1], F32, kind="ExternalInput")
    k0p2_d = nc.dram_tensor("k0p2", [128, M * O // 2], BF16,
                            kind="ExternalInput")
    k1p_d = nc.dram_tensor("k1p", [O, M * O], BF16, kind="ExternalInput")
    k2p_d = nc.dram_tensor("k2p", [O, M * O], BF16, kind="ExternalInput")
    e41_d = nc.dram_tensor("e41", [128, M1 * 8], BF16, kind="ExternalInput")
    idenb_d = nc.dram_tensor("idenb", [128, 128], BF16, kind="ExternalInput")
    idenf_d = nc.dram_tensor("idenf", [128, 128], F32, kind="ExternalInput")
    out_d = nc.dram_tensor("out", [bcl, 3 * O], F32, kind="ExternalOutput")

    with tile.TileContext(nc) as tc:
        with ExitStack() as perm:
            pp = perm.enter_context(tc.tile_pool(name="perm", bufs=1))
            x0t_sb = pp.tile([128, n_chunks * M1], F32, name="x0t_sb")
            nc.sync.dma_start(
                x0t_sb[:].rearrange("p (c m) -> p c m", m=M1),
                x0t_d.ap().rearrange("(c p) m -> p c m", p=128))
            x0tb_sb = pp.tile([128, n_chunks * M1], BF16, name="x0tb_sb")
            nc.vector.tensor_copy(x0tb_sb[:], x0t_sb[:])
            e41_sb = pp.tile([128, M1 * 8], BF16, name="e41_sb")
            nc.sync.dma_start(e41_sb[:], e41_d.ap())
            idenb_sb = pp.tile([128, 128], BF16, name="idenb_sb")
            nc.sync.dma_start(idenb_sb[:], idenb_d.ap())
            idenf_sb = pp.tile([128, 128], F32, name="idenf_sb")
            nc.sync.dma_start(idenf_sb[:], idenf_d.ap())
            k1p_sb = pp.tile([O, M * O], BF16, name="k1p_sb")
            nc.sync.dma_start(k1p_sb[:], k1p_d.ap())
            x2T_sb = pp.tile([128, bd], BF16, name="x2T_sb")
            o1_st = pp.tile([128, bcl], F32, name="o1_st")
            o2_st = pp.tile([128, bcl], F32, name="o2_st")
            o3_st = pp.tile([128, bcl], F32, name="o3_st")

            with ExitStack() as phA:
                pa = phA.enter_context(tc.tile_pool(name="phA", bufs=1))
                x0p2_sb = pa.tile([128, bd], BF16, name="x0p2_sb")
                nc.sync.dma_start(x0p2_sb[:], x0p2_d.ap())
                k0p2_sb = pa.tile([128, M * O // 2], BF16, name="k0p2_sb")
                nc.sync.dma_start(k0p2_sb[:], k0p2_d.ap())
                x1_sb = pa.tile([128, bd], BF16, name="x1_sb")
                dg_pool = phA.enter_context(tc.tile_pool(name="dgs", bufs=5))
                ts_pool = phA.enter_context(tc.tile_pool(name="tss", bufs=46))
                pt_pool = phA.enter_context(
                    tc.tile_pool(name="ptp", bufs=5, space="PSUM"))
                z_pool = phA.enter_context(
                    tc.tile_pool(name="zp", bufs=3, space="PSUM"))

                # Software pipeline: at iteration i, the PE runs (in order)
                #   L0-h(i), L0-mcontr(i-1), L1-h(i-2), L1-mcontr(i-3)
                # so every op's inputs (ts copies, x1 slices, diag sets)
                # were produced >=1 iteration earlier -- no PE stalls.
                dgs = {}        # chunk -> diag-set tile
                ts0 = {}        # chunk -> L0 t' group tiles
                ts1 = {}        # chunk -> L1 t' group tiles

                def copy_ts(pt, idx):
                    t = ts_pool.tile([128, MPG * O], BF16, name="ts",
                                     tag="ts")
                    if idx % 2 == 0:
                        nc.scalar.copy(t[:], pt[:])
                    else:
                        nc.vector.tensor_copy(t[:], pt[:])
                    return t

                for i in range(n_chunks + 3):
                    if i < n_chunks:
                        c = i
                        # all 40 diag(x0'[:,m]) for chunk c in one DVE op
                        dg = dg_pool.tile([128, M * 128], BF16, name="dg",
                                          tag="dg")
                        nc.vector.tensor_tensor(
                            dg[:].rearrange("p (m q) -> p m q", q=128),
                            x0tb_sb[:, c * M1:c * M1 + M].unsqueeze(2)
                            .broadcast_to([128, M, 128]),
                            idenb_sb[:].unsqueeze(1)
                            .broadcast_to([128, M, 128]),
                            ALU.mult)
                        dgs[c] = dg
                        # L0 h-contraction (row-packed pairs) + ts copies
                        cs = slice(c * 128, (c + 1) * 128)
                        ts0[c] = []
                        for p in range(GROUPS // 2):
                            pt_a = pt_pool.tile([128, MPG * O], F32,
                                                name="pt", tag="pt")
                            pt_b = pt_pool.tile([128, MPG * O], F32,
                                                name="pt2", tag="pt")
                            ps = slice(p * MPG * O, (p + 1) * MPG * O)
                            nc.tensor.matmul(pt_a[:], x0p2_sb[0:M, cs],
                                             k0p2_sb[0:M, ps],
                                             start=True, stop=True)
                            nc.tensor.matmul(pt_b[:], x0p2_sb[64:64 + M, cs],
                                             k0p2_sb[64:64 + M, ps],
                                             start=True, stop=True)
                            ts0[c].append(copy_ts(pt_a, 2 * p))
                            ts0[c].append(copy_ts(pt_b, 2 * p + 1))
                    if 1 <= i <= n_chunks:
                        # L0 m-contraction: zT[h,col] += t'_m^T scaled
                        c = i - 1
                        cs = slice(c * 128, (c + 1) * 128)
                        dg, tsb = dgs[c], ts0[c]
                        z0 = z_pool.tile([128, 128], F32, name="z0", tag="z")
                        for m in range(M):
                            g, j = divmod(m, MPG)
                            nc.tensor.matmul(
                                z0[:], tsb[g][:, j * O:(j + 1) * O],
                                dg[:, m * 128:(m + 1) * 128],
                                start=(m == 0), stop=(m == M - 1))
                        nc.scalar.copy(x1_sb[:, cs], z0[:])
                        del ts0[c]
                    if 2 <= i <= n_chunks + 1:
                        # L1 h-contraction + ts copies
                        c = i - 2
                        cs = slice(c * 128, (c + 1) * 128)
                        ts1[c] = []
                        for g in range(GROUPS):
                            pt = pt_pool.tile([128, MPG * O], F32,
                                              name="pt", tag="pt")
                            gs = slice(g * MPG * O, (g + 1) * MPG * O)
                            nc.tensor.matmul(pt[:], x1_sb[:, cs],
                                             k1p_sb[:, gs],
                                             start=True, stop=True)
                            ts1[c].append(copy_ts(pt, g))
                    if 3 <= i:
                        # L1 m-contraction: z[col,o] += diag_m @ t'_m
                        c = i - 3
                        cs = slice(c * 128, (c + 1) * 128)
                        dg, tsb = dgs[c], ts1[c]
                        z1 = z_pool.tile([128, 128], F32, name="z1", tag="z")
                        for m in range(M):
                            g, j = divmod(m, MPG)
                            nc.tensor.matmul(
                                z1[:], dg[:, m * 128:(m + 1) * 128],
                                tsb[g][:, j * O:(j + 1) * O],
                                start=(m == 0), stop=(m == M - 1))
                        nc.scalar.copy(x2T_sb[:, cs], z1[:])
                        del dgs[c], ts1[c]

                # out1 = sum_d x1 (strided reduce, one op)
                nc.vector.tensor_reduce(
                    o1_st[:], x1_sb[:].rearrange("p (b d) -> p b d", d=D),
                    mybir.AxisListType.X, ALU.add)

            # ---- phase B: L2 via indicator matmuls (bf16) ----
            with ExitStack() as phB:
                pb = phB.enter_context(tc.tile_pool(name="phB", bufs=1))
                k2p_sb = pb.tile([O, M * O], BF16, name="k2p_sb")
                nc.sync.dma_start(k2p_sb[:], k2p_d.ap())
                w_sb = pb.tile([128, M * bcl], BF16, name="w_sb")
                x0e_pool = phB.enter_context(tc.tile_pool(name="x0es", bufs=3))
                pw_pool = phB.enter_context(
                    tc.tile_pool(name="pwp", bufs=3, space="PSUM"))
                po3_pool = phB.enter_context(
                    tc.tile_pool(name="po3p", bufs=1, space="PSUM"))
                ptp_pool = phB.enter_context(
                    tc.tile_pool(name="ptpp", bufs=2, space="PSUM"))

                e41_3d = e41_sb[:].rearrange("p (m e) -> p m e", e=8)
                w_4d = w_sb[:].rearrange("p (m b) -> p m b", b=bcl)
                for c in range(n_chunks):
                    x0e = x0e_pool.tile([128, M1 * 8], BF16, name="x0e",
                                        tag="x0e")
                    nc.vector.tensor_tensor(
                        x0e[:].rearrange("p (m e) -> p m e", e=8),
                        x0tb_sb[:, c * M1:(c + 1) * M1].unsqueeze(2)
                        .broadcast_to([128, M1, 8]),
                        e41_3d, ALU.mult)
                    pw = pw_pool.tile([128, M1 * 8], F32, name="pw", tag="pw")
                    nc.tensor.matmul(
                        pw[:], x2T_sb[:, c * 128:(c + 1) * 128],
                        x0e[:], start=True, stop=True)
                    # scatter w part: w[h, m, c*8+j] <- pw[h, m*8+j]
                    nc.scalar.copy(
                        w_4d[:, :, c * 8:(c + 1) * 8],
                        pw[:, 0:M * 8].rearrange("p (m e) -> p m e", e=8))
                    # out2 block: columns [320:328)
                    nc.scalar.copy(o2_st[:, c * 8:(c + 1) * 8],
                                   pw[:, M * 8:M1 * 8])

                po3 = po3_pool.tile([128, bcl], F32, name="po3")
                for m in range(M):
                    nc.tensor.matmul(
                        po3[:], k2p_sb[:, m * O:(m + 1) * O],
                        w_sb[:, m * bcl:(m + 1) * bcl],
                        start=(m == 0), stop=(m == M - 1))
                nc.scalar.copy(o3_st[:], po3[:])

                # ---- outputs: transpose [o, b] tiles to [b, o] and store
                tb_pool = phB.enter_context(tc.tile_pool(name="tbs", bufs=3))
                for l, st in enumerate((o1_st, o2_st, o3_st)):
                    for j in range(nb):
                        tw = min(128, bcl - j * 128)
                        ptp = ptp_pool.tile([128, 128], F32, name="ptp",
                                            tag="ptp")
                        nc.tensor.transpose(
                            ptp[0:tw, :], st[:, j * 128:j * 128 + tw],
                            idenf_sb[:])
                        tb = tb_pool.tile([128, 128], F32, name="tb", tag="tb")
                        nc.scalar.copy(tb[0:tw, :], ptp[0:tw, :])
                        nc.sync.dma_start(
                            out_d.ap()[j * 128:j * 128 + tw,
                                       l * O:(l + 1) * O],
                            tb[0:tw, :])

    _split_excess_waits(nc)
    return nc


def host_prep(x0c, k0, k1, k2):
    """Per-core input prep. x0c: (bcl, M, D) float32."""
    bcl = x0c.shape[0]
    x0m = np.ascontiguousarray(
        x0c.transpose(1, 0, 2).reshape(M, bcl * D), dtype=np.float32)
    x0mb = x0m.astype(ml_dtypes.bfloat16)
    x0p2 = np.zeros((128, bcl * D), ml_dtypes.bfloat16)
    x0p2[0:M] = x0mb
    x0p2[64:64 + M] = x0mb
    x0t = np.concatenate(
        [x0c.transpose(0, 2, 1).reshape(bcl * D, M),
         np.ones((bcl * D, 1), np.float32)], axis=1)
    x0t = np.ascontiguousarray(x0t, dtype=np.float32)
    k0p = np.ascontiguousarray(
        k0.transpose(1, 2, 0).reshape(M, M * O), dtype=np.float32)
    k0pb = k0p.astype(ml_dtypes.bfloat16)
    npair = GROUPS // 2
    k0p2 = np.zeros((128, M * O // 2), ml_dtypes.bfloat16)
    for p in range(npair):
        ps = slice(p * MPG * O, (p + 1) * MPG * O)
        k0p2[0:M, ps] = k0pb[:, (2 * p) * MPG * O:(2 * p + 1) * MPG * O]
        k0p2[64:64 + M, ps] = k0pb[:, (2 * p + 1) * MPG * O:
                                   (2 * p + 2) * MPG * O]
    k1p = np.ascontiguousarray(
        k1.transpose(1, 2, 0).reshape(O, M * O)).astype(ml_dtypes.bfloat16)
    k2p = np.ascontiguousarray(
        k2.transpose(1, 2, 0).reshape(O, M * O)).astype(ml_dtypes.bfloat16)
    e8 = (np.arange(128)[:, None] // D == np.arange(8)[None, :])
    e8b = e8.astype(ml_dtypes.bfloat16)
    e41 = np.ascontiguousarray(
        np.tile(e8b[:, None, :], (1, M1, 1)).reshape(128, M1 * 8))
    idenb = np.eye(128, dtype=ml_dtypes.bfloat16)
    idenf = np.eye(128, dtype=np.float32)
    return {"x0p2": x0p2, "x0t": x0t, "k0p2": k0p2, "k1p": k1p,
            "k2p": k2p, "e41": e41, "idenb": idenb, "idenf": idenf}


_nc_cache = {}


def _get_nc(n_chunks):
    if n_chunks not in _nc_cache:
        _nc_cache[n_chunks] = build(n_chunks)
    return _nc_cache[n_chunks]


def kernel(x0, k0, k1, k2):
    from concourse.bass_utils import run_bass_kernel_spmd
    x0 = np.asarray(x0, dtype=np.float32)
    k0 = np.asarray(k0, dtype=np.float32)
    k1 = np.asarray(k1, dtype=np.float32)
    k2 = np.asarray(k2, dtype=np.float32)
    n_chunks = (BC * D) // 128
    nc = _get_nc(n_chunks)
    in_maps = [host_prep(x0[c * BC:(c + 1) * BC], k0, k1, k2)
               for c in range(N_CORES)]
    res = run_bass_kernel_spmd(nc, in_maps, core_ids=list(range(N_CORES)))
    out = np.concatenate([r["out"] for r in res.results], axis=0)
    return out.astype(np.float32)


# revision 19
# speedup vs baseline: 2.0762x; 1.0774x over previous
"""Trainium2 Bass kernel for nn_CIN: 3-layer compressed-interaction network.

Reference computation (per layer l with kernel k_l [O,H,M]):
    x_{l+1}[b,o,d] = sum_{h,m} x_l[b,h,d] * x0[b,m,d] * k_l[o,h,m]
    out = concat_l(sum_d x_{l+1}[b,o,d])          # (B, 3*128)

Sharding: pure data-parallel over B across 8 cores (512 batch each).

v3 ("diag-matmul"): the data-dependent m-contraction
    z[col,o] = sum_m x0'[col,m] * t'_m[col,o]
runs entirely on the TensorEngine as 40 PSUM-accumulated matmuls per
chunk-layer against materialized diagonal matrices diag(x0'[:,m]).
All 40 bf16 diagonals for a chunk are built in ONE wide DVE op
(x0-broadcast times an identity mask) and shared by L0 and L1 (the
chunk loops are fused).  Two matmul orientations avoid all transposes:
  L0: lhsT = t'_m [col,h-block] (stationary), rhs = diag_m
      -> out zT[h, col] accumulated over m = x1 orientation directly.
  L1: lhsT = diag_m (stationary), rhs = t'_m [col,o]
      -> out z[col, o] = x2T orientation directly.
t' tiles move PSUM->SBUF (bf16 downcast) via ACT/DVE copies split
between the two engines.  out1 = one strided DVE reduce over x1.
Everything matmul is single bf16 (tolerance 2e-2 >> bf16 error ~4e-3).

  L2 (d-sum folded into PE): out3[b,o] = sum_{hm} k2[o,h,m] * w[b,h,m]
  with w[b,h,m] = sum_d x2[b,h,d]*x0[b,m,d].  Per chunk, one bf16 matmul
  lhsT=x2T_c[col,h], rhs=X0E[col,(m,b')] where X0E = x0'[col,m]*E[col,b']
  (E = batch indicator) contracts col, yielding w[h,(m,b')] plus, via an
  appended plain-indicator block, the d-sum of x2 (=out2).  A final
  40-matmul PSUM accumulation k2p_m @ w_m produces out3[o,b].
"""

import os
import numpy as np
from contextlib import ExitStack

import ml_dtypes
import concourse.bass as bass
import concourse.tile as tile
import concourse.mybir as mybir

# The stock compile pipeline passes --enable-ldw-opt=false to walrus.
# Flipping it crashes walrus codegen (visitInstLdweights) in this
# environment, so leave it off unless CIN_LDW_OPT=1 is set explicitly.
if os.environ.get("CIN_LDW_OPT", "0") == "1":
    import concourse.bass_utils as _bu

    if not getattr(_bu, "_cin_ldw_patched", False):
        _orig_run_command = _bu.run_command

        def _run_command_ldw(cmd, *args, **kwargs):
            if isinstance(cmd, (list, tuple)):
                cmd = [c.replace("--enable-ldw-opt=false",
                                 "--enable-ldw-opt=true")
                       if isinstance(c, str) else c for c in cmd]
            return _orig_run_command(cmd, *args, **kwargs)

        _bu.run_command = _run_command_ldw
        _bu._cin_ldw_patched = True

F32 = mybir.dt.float32
BF16 = mybir.dt.bfloat16
ALU = mybir.AluOpType
AFT = mybir.ActivationFunctionType

B, M, D, O = 4096, 40, 16, 128
N_CORES = 8
BC = B // N_CORES          # 512 batch rows per core
MPG = 4                    # m's packed per matmul (4*128 = 512 free cols)
GROUPS = M // MPG          # 10
M1 = M + 1                 # x0t carries a trailing ones-column

_ns_ctr = [0]


def _split_excess_waits(nc, max_waits=1):
    """walrus in this env rejects >1 sync-wait on one instruction (CTRL
    struct): move excess waits onto same-engine NoOps inserted before."""
    for f in nc.m.functions:
        for bb in f.blocks:
            new_list = []
            for inst in bb.instructions:
                si = inst.sync_info
                waits = list(si.on_wait) if si and si.on_wait else []
                if len(waits) > max_waits:
                    excess = waits[:-max_waits]
                    keep = waits[-max_waits:]
                    for i in range(0, len(excess), max_waits):
                        chunk = excess[i:i + max_waits]
                        _ns_ctr[0] += 1
                        nop = mybir.InstNoOp(
                            name=f"waitsplit-{_ns_ctr[0]}", ins=[], outs=[],
                            engine=inst.engine,
                            sync_info=mybir.SyncInfo(on_wait=chunk, on_update=[]),
                        )
                        nc.register_instruction(nop)
                        new_list.append(nop)
                    si.on_wait = keep
                    inst.sync_info = si
                new_list.append(inst)
            bb.instructions[:] = new_list


def build(n_chunks):
    """Build the per-core Bass program for n_chunks*128 bd columns."""
    bd = n_chunks * 128
    bcl = bd // D              # local batch count
    nb = (bcl + 127) // 128    # output b-tiles
    nc = bass.Bass("TRN2", target_bir_lowering=False, debug=False, num_devices=1)

    x0p2_d = nc.dram_tensor("x0p2", [128, bd], BF16, kind="ExternalInput")
    x0t_d = nc.dram_tensor("x0t", [bd, M1], F32, kind="ExternalInput")
    k0p2_d = nc.dram_tensor("k0p2", [128, M * O // 2], BF16,
                            kind="ExternalInput")
    k1p_d = nc.dram_tensor("k1p", [O, M * O], BF16, kind="ExternalInput")
    k2p_d = nc.dram_tensor("k2p", [O, M * O], BF16, kind="ExternalInput")
    e41_d = nc.dram_tensor("e41", [128, M1 * 8], BF16, kind="ExternalInput")
    idenb_d = nc.dram_tensor("idenb", [128, 128], BF16, kind="ExternalInput")
    idenf_d = nc.dram_tensor("idenf", [128, 128], F32, kind="ExternalInput")
    out_d = nc.dram_tensor("out", [bcl, 3 * O], F32, kind="ExternalOutput")

    with tile.TileContext(nc) as tc:
        with ExitStack() as perm:
            pp = perm.enter_context(tc.tile_pool(name="perm", bufs=1))
            x0t_sb = pp.tile([128, n_chunks * M1], F32, name="x0t_sb")
            nc.sync.dma_start(
                x0t_sb[:].rearrange("p (c m) -> p c m", m=M1),
                x0t_d.ap().rearrange("(c p) m -> p c m", p=128))
            x0tb_sb = pp.tile([128, n_chunks * M1], BF16, name="x0tb_sb")
            nc.vector.tensor_copy(x0tb_sb[:], x0t_sb[:])
            e41_sb = pp.tile([128, M1 * 8], BF16, name="e41_sb")
            nc.sync.dma_start(e41_sb[:], e41_d.ap())
            idenb_sb = pp.tile([128, 128], BF16, name="idenb_sb")
            nc.sync.dma_start(idenb_sb[:], idenb_d.ap())
            idenf_sb = pp.tile([128, 128], F32, name="idenf_sb")
            nc.sync.dma_start(idenf_sb[:], idenf_d.ap())
            k1p_sb = pp.tile([O, M * O], BF16, name="k1p_sb")
            nc.sync.dma_start(k1p_sb[:], k1p_d.ap())
            x2T_sb = pp.tile([128, bd], BF16, name="x2T_sb")
            o1_st = pp.tile([128, bcl], F32, name="o1_st")
            o2_st = pp.tile([128, bcl], F32, name="o2_st")
            o3_st = pp.tile([128, bcl], F32, name="o3_st")

            with ExitStack() as phA:
                pa = phA.enter_context(tc.tile_pool(name="phA", bufs=1))
                x0p2_sb = pa.tile([128, bd], BF16, name="x0p2_sb")
                nc.sync.dma_start(x0p2_sb[:], x0p2_d.ap())
                k0p2_sb = pa.tile([128, M * O // 2], BF16, name="k0p2_sb")
                nc.sync.dma_start(k0p2_sb[:], k0p2_d.ap())
                x1_sb = pa.tile([128, bd], BF16, name="x1_sb")
                dg_pool = phA.enter_context(tc.tile_pool(name="dgs", bufs=5))
                ts_pool = phA.enter_context(tc.tile_pool(name="tss", bufs=46))
                pt_pool = phA.enter_context(
                    tc.tile_pool(name="ptp", bufs=5, space="PSUM"))
                z_pool = phA.enter_context(
                    tc.tile_pool(name="zp", bufs=3, space="PSUM"))

                # Software pipeline: at iteration i, the PE runs (in order)
                #   L0-h(i), L0-mcontr(i-1), L1-h(i-2), L1-mcontr(i-3)
                # so every op's inputs (ts copies, x1 slices, diag sets)
                # were produced >=1 iteration earlier -- no PE stalls.
                dgs = {}        # chunk -> diag-set tile
                ts0 = {}        # chunk -> L0 t' group tiles
                ts1 = {}        # chunk -> L1 t' group tiles

                def copy_ts(pt, idx):
                    t = ts_pool.tile([128, MPG * O], BF16, name="ts",
                                     tag="ts")
                    if idx % 3 != 2:       # 2/3 on ACT, 1/3 on DVE
                        nc.scalar.copy(t[:], pt[:])
                    else:
                        nc.vector.tensor_copy(t[:], pt[:])
                    return t

                for i in range(n_chunks + 3):
                    if i < n_chunks:
                        c = i
                        # all 40 diag(x0'[:,m]) for chunk c: half the set
                        # on DVE, half on (otherwise idle) GPSIMD
                        dg = dg_pool.tile([128, M * 128], BF16, name="dg",
                                          tag="dg")
                        MH = M // 2
                        nc.vector.tensor_tensor(
                            dg[:, 0:MH * 128]
                            .rearrange("p (m q) -> p m q", q=128),
                            x0tb_sb[:, c * M1:c * M1 + MH].unsqueeze(2)
                            .broadcast_to([128, MH, 128]),
                            idenb_sb[:].unsqueeze(1)
                            .broadcast_to([128, MH, 128]),
                            ALU.mult)
                        nc.gpsimd.tensor_tensor(
                            dg[:, MH * 128:M * 128]
                            .rearrange("p (m q) -> p m q", q=128),
                            x0tb_sb[:, c * M1 + MH:c * M1 + M].unsqueeze(2)
                            .broadcast_to([128, M - MH, 128]),
                            idenb_sb[:].unsqueeze(1)
                            .broadcast_to([128, M - MH, 128]),
                            ALU.mult)
                        dgs[c] = dg
                        # L0 h-contraction (row-packed pairs) + ts copies
                        cs = slice(c * 128, (c + 1) * 128)
                        ts0[c] = []
                        for p in range(GROUPS // 2):
                            pt_a = pt_pool.tile([128, MPG * O], F32,
                                                name="pt", tag="pt")
                            pt_b = pt_pool.tile([128, MPG * O], F32,
                                                name="pt2", tag="pt")
                            ps = slice(p * MPG * O, (p + 1) * MPG * O)
                            nc.tensor.matmul(pt_a[:], x0p2_sb[0:M, cs],
                                             k0p2_sb[0:M, ps],
                                             start=True, stop=True)
                            nc.tensor.matmul(pt_b[:], x0p2_sb[64:64 + M, cs],
                                             k0p2_sb[64:64 + M, ps],
                                             start=True, stop=True)
                            ts0[c].append(copy_ts(pt_a, 2 * p))
                            ts0[c].append(copy_ts(pt_b, 2 * p + 1))
                    if 1 <= i <= n_chunks:
                        # L0 m-contraction: zT[h,col] += t'_m^T scaled
                        c = i - 1
                        cs = slice(c * 128, (c + 1) * 128)
                        dg, tsb = dgs[c], ts0[c]
                        z0 = z_pool.tile([128, 128], F32, name="z0", tag="z")
                        for m in range(M):
                            g, j = divmod(m, MPG)
                            nc.tensor.matmul(
                                z0[:], tsb[g][:, j * O:(j + 1) * O],
                                dg[:, m * 128:(m + 1) * 128],
                                start=(m == 0), stop=(m == M - 1))
                        nc.scalar.copy(x1_sb[:, cs], z0[:])
                        del ts0[c]
                    if 2 <= i <= n_chunks + 1:
                        # L1 h-contraction + ts copies
                        c = i - 2
                        cs = slice(c * 128, (c + 1) * 128)
                        ts1[c] = []
                        for g in range(GROUPS):
                            pt = pt_pool.tile([128, MPG * O], F32,
                                              name="pt", tag="pt")
                            gs = slice(g * MPG * O, (g + 1) * MPG * O)
                            nc.tensor.matmul(pt[:], x1_sb[:, cs],
                                             k1p_sb[:, gs],
                                             start=True, stop=True)
                            ts1[c].append(copy_ts(pt, g))
                    if 3 <= i:
                        # L1 m-contraction: z[col,o] += diag_m @ t'_m
                        c = i - 3
                        cs = slice(c * 128, (c + 1) * 128)
                        dg, tsb = dgs[c], ts1[c]
                        z1 = z_pool.tile([128, 128], F32, name="z1", tag="z")
                        for m in range(M):
                            g, j = divmod(m, MPG)
                            nc.tensor.matmul(
                                z1[:], dg[:, m * 128:(m + 1) * 128],
                                tsb[g][:, j * O:(j + 1) * O],
                                start=(m == 0), stop=(m == M - 1))
                        nc.scalar.copy(x2T_sb[:, cs], z1[:])
                        del dgs[c], ts1[c]

                # out1 = sum_d x1 (strided reduce, one op)
                nc.vector.tensor_reduce(
                    o1_st[:], x1_sb[:].rearrange("p (b d) -> p b d", d=D),
                    mybir.AxisListType.X, ALU.add)

            # ---- phase B: L2 via indicator matmuls (bf16) ----
            with ExitStack() as phB:
                pb = phB.enter_context(tc.tile_pool(name="phB", bufs=1))
                k2p_sb = pb.tile([O, M * O], BF16, name="k2p_sb")
                nc.sync.dma_start(k2p_sb[:], k2p_d.ap())
                w_sb = pb.tile([128, M * bcl], BF16, name="w_sb")
                x0e_pool = phB.enter_context(tc.tile_pool(name="x0es", bufs=3))
                pw_pool = phB.enter_context(
                    tc.tile_pool(name="pwp", bufs=3, space="PSUM"))
                po3_pool = phB.enter_context(
                    tc.tile_pool(name="po3p", bufs=1, space="PSUM"))
                ptp_pool = phB.enter_context(
                    tc.tile_pool(name="ptpp", bufs=2, space="PSUM"))

                e41_3d = e41_sb[:].rearrange("p (m e) -> p m e", e=8)
                w_4d = w_sb[:].rearrange("p (m b) -> p m b", b=bcl)
                for c in range(n_chunks):
                    x0e = x0e_pool.tile([128, M1 * 8], BF16, name="x0e",
                                        tag="x0e")
                    nc.vector.tensor_tensor(
                        x0e[:].rearrange("p (m e) -> p m e", e=8),
                        x0tb_sb[:, c * M1:(c + 1) * M1].unsqueeze(2)
                        .broadcast_to([128, M1, 8]),
                        e41_3d, ALU.mult)
                    pw = pw_pool.tile([128, M1 * 8], F32, name="pw", tag="pw")
                    nc.tensor.matmul(
                        pw[:], x2T_sb[:, c * 128:(c + 1) * 128],
                        x0e[:], start=True, stop=True)
                    # scatter w part: w[h, m, c*8+j] <- pw[h, m*8+j]
                    nc.scalar.copy(
                        w_4d[:, :, c * 8:(c + 1) * 8],
                        pw[:, 0:M * 8].rearrange("p (m e) -> p m e", e=8))
                    # out2 block: columns [320:328)
                    nc.scalar.copy(o2_st[:, c * 8:(c + 1) * 8],
                                   pw[:, M * 8:M1 * 8])

                po3 = po3_pool.tile([128, bcl], F32, name="po3")
                for m in range(M):
                    nc.tensor.matmul(
                        po3[:], k2p_sb[:, m * O:(m + 1) * O],
                        w_sb[:, m * bcl:(m + 1) * bcl],
                        start=(m == 0), stop=(m == M - 1))
                nc.scalar.copy(o3_st[:], po3[:])

                # ---- outputs: transpose [o, b] tiles to [b, o] and store
                tb_pool = phB.enter_context(tc.tile_pool(name="tbs", bufs=3))
                for l, st in enumerate((o1_st, o2_st, o3_st)):
                    for j in range(nb):
                        tw = min(128, bcl - j * 128)
                        ptp = ptp_pool.tile([128, 128], F32, name="ptp",
                                            tag="ptp")
                        nc.tensor.transpose(
                            ptp[0:tw, :], st[:, j * 128:j * 128 + tw],
                            idenf_sb[:])
                        tb = tb_pool.tile([128, 128], F32, name="tb", tag="tb")
                        nc.scalar.copy(tb[0:tw, :], ptp[0:tw, :])
                        nc.sync.dma_start(
                            out_d.ap()[j * 128:j * 128 + tw,
                                       l * O:(l + 1) * O],
                            tb[0:tw, :])

    _split_excess_waits(nc)
    return nc


def host_prep(x0c, k0, k1, k2):
    """Per-core input prep. x0c: (bcl, M, D) float32."""
    bcl = x0c.shape[0]
    x0m = np.ascontiguousarray(
        x0c.transpose(1, 0, 2).reshape(M, bcl * D), dtype=np.float32)
    x0mb = x0m.astype(ml_dtypes.bfloat16)
    x0p2 = np.zeros((128, bcl * D), ml_dtypes.bfloat16)
    x0p2[0:M] = x0mb
    x0p2[64:64 + M] = x0mb
    x0t = np.concatenate(
        [x0c.transpose(0, 2, 1).reshape(bcl * D, M),
         np.ones((bcl * D, 1), np.float32)], axis=1)
    x0t = np.ascontiguousarray(x0t, dtype=np.float32)
    k0p = np.ascontiguousarray(
        k0.transpose(1, 2, 0).reshape(M, M * O), dtype=np.float32)
    k0pb = k0p.astype(ml_dtypes.bfloat16)
    npair = GROUPS // 2
    k0p2 = np.zeros((128, M * O // 2), ml_dtypes.bfloat16)
    for p in range(npair):
        ps = slice(p * MPG * O, (p + 1) * MPG * O)
        k0p2[0:M, ps] = k0pb[:, (2 * p) * MPG * O:(2 * p + 1) * MPG * O]
        k0p2[64:64 + M, ps] = k0pb[:, (2 * p + 1) * MPG * O:
                                   (2 * p + 2) * MPG * O]
    k1p = np.ascontiguousarray(
        k1.transpose(1, 2, 0).reshape(O, M * O)).astype(ml_dtypes.bfloat16)
    k2p = np.ascontiguousarray(
        k2.transpose(1, 2, 0).reshape(O, M * O)).astype(ml_dtypes.bfloat16)
    e8 = (np.arange(128)[:, None] // D == np.arange(8)[None, :])
    e8b = e8.astype(ml_dtypes.bfloat16)
    e41 = np.ascontiguousarray(
        np.tile(e8b[:, None, :], (1, M1, 1)).reshape(128, M1 * 8))
    idenb = np.eye(128, dtype=ml_dtypes.bfloat16)
    idenf = np.eye(128, dtype=np.float32)
    return {"x0p2": x0p2, "x0t": x0t, "k0p2": k0p2, "k1p": k1p,
            "k2p": k2p, "e41": e41, "idenb": idenb, "idenf": idenf}


_nc_cache = {}


def _get_nc(n_chunks):
    if n_chunks not in _nc_cache:
        _nc_cache[n_chunks] = build(n_chunks)
    return _nc_cache[n_chunks]


def kernel(x0, k0, k1, k2):
    from concourse.bass_utils import run_bass_kernel_spmd
    x0 = np.asarray(x0, dtype=np.float32)
    k0 = np.asarray(k0, dtype=np.float32)
    k1 = np.asarray(k1, dtype=np.float32)
    k2 = np.asarray(k2, dtype=np.float32)
    n_chunks = (BC * D) // 128
    nc = _get_nc(n_chunks)
    in_maps = [host_prep(x0[c * BC:(c + 1) * BC], k0, k1, k2)
               for c in range(N_CORES)]
    res = run_bass_kernel_spmd(nc, in_maps, core_ids=list(range(N_CORES)))
    out = np.concatenate([r["out"] for r in res.results], axis=0)
    return out.astype(np.float32)


# revision 22
# speedup vs baseline: 2.1475x; 1.0344x over previous
"""Trainium2 Bass kernel for nn_CIN: 3-layer compressed-interaction network.

Reference computation (per layer l with kernel k_l [O,H,M]):
    x_{l+1}[b,o,d] = sum_{h,m} x_l[b,h,d] * x0[b,m,d] * k_l[o,h,m]
    out = concat_l(sum_d x_{l+1}[b,o,d])          # (B, 3*128)

Sharding: pure data-parallel over B across 8 cores (512 batch each).

v3 ("diag-matmul"): the data-dependent m-contraction
    z[col,o] = sum_m x0'[col,m] * t'_m[col,o]
runs entirely on the TensorEngine as 40 PSUM-accumulated matmuls per
chunk-layer against materialized diagonal matrices diag(x0'[:,m]).
All 40 bf16 diagonals for a chunk are built in ONE wide DVE op
(x0-broadcast times an identity mask) and shared by L0 and L1 (the
chunk loops are fused).  Two matmul orientations avoid all transposes:
  L0: lhsT = t'_m [col,h-block] (stationary), rhs = diag_m
      -> out zT[h, col] accumulated over m = x1 orientation directly.
  L1: lhsT = diag_m (stationary), rhs = t'_m [col,o]
      -> out z[col, o] = x2T orientation directly.
t' tiles move PSUM->SBUF (bf16 downcast) via ACT/DVE copies split
between the two engines.  out1 = one strided DVE reduce over x1.
Everything matmul is single bf16 (tolerance 2e-2 >> bf16 error ~4e-3).

  L2 (d-sum folded into PE): out3[b,o] = sum_{hm} k2[o,h,m] * w[b,h,m]
  with w[b,h,m] = sum_d x2[b,h,d]*x0[b,m,d].  Per chunk, one bf16 matmul
  lhsT=x2T_c[col,h], rhs=X0E[col,(m,b')] where X0E = x0'[col,m]*E[col,b']
  (E = batch indicator) contracts col, yielding w[h,(m,b')] plus, via an
  appended plain-indicator block, the d-sum of x2 (=out2).  A final
  40-matmul PSUM accumulation k2p_m @ w_m produces out3[o,b].
"""

import os
import numpy as np
from contextlib import ExitStack

import ml_dtypes
import concourse.bass as bass
import concourse.tile as tile
import concourse.mybir as mybir

# The stock compile pipeline passes --enable-ldw-opt=false to walrus.
# Flipping it crashes walrus codegen (visitInstLdweights) in this
# environment, so leave it off unless CIN_LDW_OPT=1 is set explicitly.
if os.environ.get("CIN_LDW_OPT", "0") == "1":
    import concourse.bass_utils as _bu

    if not getattr(_bu, "_cin_ldw_patched", False):
        _orig_run_command = _bu.run_command

        def _run_command_ldw(cmd, *args, **kwargs):
            if isinstance(cmd, (list, tuple)):
                cmd = [c.replace("--enable-ldw-opt=false",
                                 "--enable-ldw-opt=true")
                       if isinstance(c, str) else c for c in cmd]
            return _orig_run_command(cmd, *args, **kwargs)

        _bu.run_command = _run_command_ldw
        _bu._cin_ldw_patched = True

F32 = mybir.dt.float32
BF16 = mybir.dt.bfloat16
ALU = mybir.AluOpType
AFT = mybir.ActivationFunctionType

B, M, D, O = 4096, 40, 16, 128
N_CORES = 8
BC = B // N_CORES          # 512 batch rows per core
MPG = 4                    # m's packed per matmul (4*128 = 512 free cols)
GROUPS = M // MPG          # 10
M1 = M + 1                 # x0t carries a trailing ones-column

_ns_ctr = [0]


def _split_excess_waits(nc, max_waits=1):
    """walrus in this env rejects >1 sync-wait on one instruction (CTRL
    struct): move excess waits onto same-engine NoOps inserted before."""
    for f in nc.m.functions:
        for bb in f.blocks:
            new_list = []
            for inst in bb.instructions:
                si = inst.sync_info
                waits = list(si.on_wait) if si and si.on_wait else []
                if len(waits) > max_waits:
                    excess = waits[:-max_waits]
                    keep = waits[-max_waits:]
                    for i in range(0, len(excess), max_waits):
                        chunk = excess[i:i + max_waits]
                        _ns_ctr[0] += 1
                        nop = mybir.InstNoOp(
                            name=f"waitsplit-{_ns_ctr[0]}", ins=[], outs=[],
                            engine=inst.engine,
                            sync_info=mybir.SyncInfo(on_wait=chunk, on_update=[]),
                        )
                        nc.register_instruction(nop)
                        new_list.append(nop)
                    si.on_wait = keep
                    inst.sync_info = si
                new_list.append(inst)
            bb.instructions[:] = new_list


def build(n_chunks):
    """Build the per-core Bass program for n_chunks*128 bd columns."""
    bd = n_chunks * 128
    bcl = bd // D              # local batch count
    nb = (bcl + 127) // 128    # output b-tiles
    nc = bass.Bass("TRN2", target_bir_lowering=False, debug=False, num_devices=1)

    x0p2_d = nc.dram_tensor("x0p2", [128, bd], BF16, kind="ExternalInput")
    x0t_d = nc.dram_tensor("x0t", [bd, M1], F32, kind="ExternalInput")
    k0p2_d = nc.dram_tensor("k0p2", [128, M * O // 2], BF16,
                            kind="ExternalInput")
    k1p_d = nc.dram_tensor("k1p", [O, M * O], BF16, kind="ExternalInput")
    k2p_d = nc.dram_tensor("k2p", [O, M * O], BF16, kind="ExternalInput")
    e41_d = nc.dram_tensor("e41", [128, M1 * 8], BF16, kind="ExternalInput")
    idenb_d = nc.dram_tensor("idenb", [128, 128], BF16, kind="ExternalInput")
    idenf_d = nc.dram_tensor("idenf", [128, 128], F32, kind="ExternalInput")
    out_d = nc.dram_tensor("out", [bcl, 3 * O], F32, kind="ExternalOutput")

    with tile.TileContext(nc) as tc:
        with ExitStack() as perm:
            pp = perm.enter_context(tc.tile_pool(name="perm", bufs=1))
            x0t_sb = pp.tile([128, n_chunks * M1], F32, name="x0t_sb")
            nc.sync.dma_start(
                x0t_sb[:].rearrange("p (c m) -> p c m", m=M1),
                x0t_d.ap().rearrange("(c p) m -> p c m", p=128))
            x0tb_sb = pp.tile([128, n_chunks * M1], BF16, name="x0tb_sb")
            nc.vector.tensor_copy(x0tb_sb[:], x0t_sb[:])
            e41_sb = pp.tile([128, M1 * 8], BF16, name="e41_sb")
            nc.sync.dma_start(e41_sb[:], e41_d.ap())
            idenb_sb = pp.tile([128, 128], BF16, name="idenb_sb")
            nc.sync.dma_start(idenb_sb[:], idenb_d.ap())
            idenf_sb = pp.tile([128, 128], F32, name="idenf_sb")
            nc.sync.dma_start(idenf_sb[:], idenf_d.ap())
            k1p_sb = pp.tile([O, M * O], BF16, name="k1p_sb")
            nc.sync.dma_start(k1p_sb[:], k1p_d.ap())
            x2T_sb = pp.tile([128, bd], BF16, name="x2T_sb")
            o1_st = pp.tile([128, bcl], F32, name="o1_st")
            o2_st = pp.tile([128, bcl], F32, name="o2_st")
            o3_st = pp.tile([128, bcl], F32, name="o3_st")

            with ExitStack() as phA:
                pa = phA.enter_context(tc.tile_pool(name="phA", bufs=1))
                x0p2_sb = pa.tile([128, bd], BF16, name="x0p2_sb")
                nc.sync.dma_start(x0p2_sb[:], x0p2_d.ap())
                k0p2_sb = pa.tile([128, M * O // 2], BF16, name="k0p2_sb")
                nc.sync.dma_start(k0p2_sb[:], k0p2_d.ap())
                x1_sb = pa.tile([128, bd], BF16, name="x1_sb")
                dg_pool = phA.enter_context(tc.tile_pool(name="dgs", bufs=5))
                ts_pool = phA.enter_context(tc.tile_pool(name="tss", bufs=46))
                pt_pool = phA.enter_context(
                    tc.tile_pool(name="ptp", bufs=5, space="PSUM"))
                z_pool = phA.enter_context(
                    tc.tile_pool(name="zp", bufs=3, space="PSUM"))

                # Software pipeline: at iteration i, the PE runs (in order)
                #   L0-h(i), L0-mcontr(i-1), L1-h(i-2), L1-mcontr(i-3)
                # so every op's inputs (ts copies, x1 slices, diag sets)
                # were produced >=1 iteration earlier -- no PE stalls.
                dgs = {}        # chunk -> diag-set tile
                ts0 = {}        # chunk -> L0 t' group tiles
                ts1 = {}        # chunk -> L1 t' group tiles
                acc1s = {}      # chunk -> L1 DVE-path accumulator
                acc_pool = phA.enter_context(
                    tc.tile_pool(name="accs", bufs=3))

                def copy_ts(pt, idx):
                    t = ts_pool.tile([128, MPG * O], BF16, name="ts",
                                     tag="ts")
                    if idx % 3 != 2:       # 2/3 on ACT, 1/3 on DVE
                        nc.scalar.copy(t[:], pt[:])
                    else:
                        nc.vector.tensor_copy(t[:], pt[:])
                    return t

                for i in range(n_chunks + 3):
                    if i < n_chunks:
                        c = i
                        # all 40 diag(x0'[:,m]) for chunk c: half the set
                        # on DVE, half on (otherwise idle) GPSIMD
                        dg = dg_pool.tile([128, M * 128], BF16, name="dg",
                                          tag="dg")
                        MH = M // 2
                        nc.vector.tensor_tensor(
                            dg[:, 0:MH * 128]
                            .rearrange("p (m q) -> p m q", q=128),
                            x0tb_sb[:, c * M1:c * M1 + MH].unsqueeze(2)
                            .broadcast_to([128, MH, 128]),
                            idenb_sb[:].unsqueeze(1)
                            .broadcast_to([128, MH, 128]),
                            ALU.mult)
                        nc.gpsimd.tensor_tensor(
                            dg[:, MH * 128:M * 128]
                            .rearrange("p (m q) -> p m q", q=128),
                            x0tb_sb[:, c * M1 + MH:c * M1 + M].unsqueeze(2)
                            .broadcast_to([128, M - MH, 128]),
                            idenb_sb[:].unsqueeze(1)
                            .broadcast_to([128, M - MH, 128]),
                            ALU.mult)
                        dgs[c] = dg
                        # L0 h-contraction (row-packed pairs) + ts copies
                        cs = slice(c * 128, (c + 1) * 128)
                        ts0[c] = []
                        for p in range(GROUPS // 2):
                            pt_a = pt_pool.tile([128, MPG * O], F32,
                                                name="pt", tag="pt")
                            pt_b = pt_pool.tile([128, MPG * O], F32,
                                                name="pt2", tag="pt")
                            ps = slice(p * MPG * O, (p + 1) * MPG * O)
                            nc.tensor.matmul(pt_a[:], x0p2_sb[0:M, cs],
                                             k0p2_sb[0:M, ps],
                                             start=True, stop=True)
                            nc.tensor.matmul(pt_b[:], x0p2_sb[64:64 + M, cs],
                                             k0p2_sb[64:64 + M, ps],
                                             start=True, stop=True)
                            ts0[c].append(copy_ts(pt_a, 2 * p))
                            ts0[c].append(copy_ts(pt_b, 2 * p + 1))
                    if 1 <= i <= n_chunks:
                        # L0 m-contraction: zT[h,col] += t'_m^T scaled
                        c = i - 1
                        cs = slice(c * 128, (c + 1) * 128)
                        dg, tsb = dgs[c], ts0[c]
                        z0 = z_pool.tile([128, 128], F32, name="z0", tag="z")
                        for m in range(M):
                            g, j = divmod(m, MPG)
                            nc.tensor.matmul(
                                z0[:], tsb[g][:, j * O:(j + 1) * O],
                                dg[:, m * 128:(m + 1) * 128],
                                start=(m == 0), stop=(m == M - 1))
                        nc.scalar.copy(x1_sb[:, cs], z0[:])
                        del ts0[c]
                    if 2 <= i <= n_chunks + 1:
                        # L1 h-contraction; groups 0..7 -> ts copies for
                        # PE diag-MMs, groups 8,9 -> DVE fused STT straight
                        # from PSUM into acc1 (uses DVE slack, saves 8
                        # LDW-bound diag matmuls per chunk)
                        c = i - 2
                        cs = slice(c * 128, (c + 1) * 128)
                        ts1[c] = []
                        acc1 = acc_pool.tile([128, 128], F32, name="acc1",
                                             tag="acc1")
                        acc1s[c] = acc1
                        first = True
                        for g in range(GROUPS):
                            pt = pt_pool.tile([128, MPG * O], F32,
                                              name="pt", tag="pt")
                            gs = slice(g * MPG * O, (g + 1) * MPG * O)
                            nc.tensor.matmul(pt[:], x1_sb[:, cs],
                                             k1p_sb[:, gs],
                                             start=True, stop=True)
                            if g < GROUPS - 2:
                                ts1[c].append(copy_ts(pt, g))
                            else:
                                for j in range(MPG):
                                    m = g * MPG + j
                                    sc = x0t_sb[:, c * M1 + m:c * M1 + m + 1]
                                    src = pt[:, j * O:(j + 1) * O]
                                    if first:
                                        nc.vector.tensor_scalar(
                                            acc1[:], src, sc, None, ALU.mult)
                                        first = False
                                    else:
                                        nc.vector.scalar_tensor_tensor(
                                            acc1[:], src, sc, acc1[:],
                                            ALU.mult, ALU.add)
                    if 3 <= i:
                        # L1 m-contraction: z[col,o] += diag_m @ t'_m for
                        # m 0..31; merge PSUM z with acc1 into x2T (DVE)
                        c = i - 3
                        cs = slice(c * 128, (c + 1) * 128)
                        dg, tsb = dgs[c], ts1[c]
                        z1 = z_pool.tile([128, 128], F32, name="z1", tag="z")
                        for m in range((GROUPS - 2) * MPG):
                            g, j = divmod(m, MPG)
                            nc.tensor.matmul(
                                z1[:], dg[:, m * 128:(m + 1) * 128],
                                tsb[g][:, j * O:(j + 1) * O],
                                start=(m == 0),
                                stop=(m == (GROUPS - 2) * MPG - 1))
                        nc.vector.tensor_tensor(
                            x2T_sb[:, cs], z1[:], acc1s[c][:], ALU.add)
                        del dgs[c], ts1[c], acc1s[c]

                # out1 = sum_d x1 (strided reduce, one op)
                nc.vector.tensor_reduce(
                    o1_st[:], x1_sb[:].rearrange("p (b d) -> p b d", d=D),
                    mybir.AxisListType.X, ALU.add)

            # ---- phase B: L2 via indicator matmuls (bf16) ----
            with ExitStack() as phB:
                pb = phB.enter_context(tc.tile_pool(name="phB", bufs=1))
                k2p_sb = pb.tile([O, M * O], BF16, name="k2p_sb")
                nc.sync.dma_start(k2p_sb[:], k2p_d.ap())
                w_sb = pb.tile([128, M * bcl], BF16, name="w_sb")
                x0e_pool = phB.enter_context(tc.tile_pool(name="x0es", bufs=3))
                pw_pool = phB.enter_context(
                    tc.tile_pool(name="pwp", bufs=3, space="PSUM"))
                po3_pool = phB.enter_context(
                    tc.tile_pool(name="po3p", bufs=1, space="PSUM"))
                ptp_pool = phB.enter_context(
                    tc.tile_pool(name="ptpp", bufs=2, space="PSUM"))

                e41_3d = e41_sb[:].rearrange("p (m e) -> p m e", e=8)
                w_4d = w_sb[:].rearrange("p (m b) -> p m b", b=bcl)
                for c in range(n_chunks):
                    x0e = x0e_pool.tile([128, M1 * 8], BF16, name="x0e",
                                        tag="x0e")
                    eng = nc.vector if c % 2 == 0 else nc.gpsimd
                    eng.tensor_tensor(
                        x0e[:].rearrange("p (m e) -> p m e", e=8),
                        x0tb_sb[:, c * M1:(c + 1) * M1].unsqueeze(2)
                        .broadcast_to([128, M1, 8]),
                        e41_3d, ALU.mult)
                    pw = pw_pool.tile([128, M1 * 8], F32, name="pw", tag="pw")
                    nc.tensor.matmul(
                        pw[:], x2T_sb[:, c * 128:(c + 1) * 128],
                        x0e[:], start=True, stop=True)
                    # scatter w part: w[h, m, c*8+j] <- pw[h, m*8+j]
                    nc.scalar.copy(
                        w_4d[:, :, c * 8:(c + 1) * 8],
                        pw[:, 0:M * 8].rearrange("p (m e) -> p m e", e=8))
                    # out2 block: columns [320:328)
                    nc.scalar.copy(o2_st[:, c * 8:(c + 1) * 8],
                                   pw[:, M * 8:M1 * 8])

                po3 = po3_pool.tile([128, bcl], F32, name="po3")
                for m in range(M):
                    nc.tensor.matmul(
                        po3[:], k2p_sb[:, m * O:(m + 1) * O],
                        w_sb[:, m * bcl:(m + 1) * bcl],
                        start=(m == 0), stop=(m == M - 1))
                nc.scalar.copy(o3_st[:], po3[:])

                # ---- outputs: transpose [o, b] tiles to [b, o] and store
                tb_pool = phB.enter_context(tc.tile_pool(name="tbs", bufs=3))
                for l, st in enumerate((o1_st, o2_st, o3_st)):
                    for j in range(nb):
                        tw = min(128, bcl - j * 128)
                        ptp = ptp_pool.tile([128, 128], F32, name="ptp",
                                            tag="ptp")
                        nc.tensor.transpose(
                            ptp[0:tw, :], st[:, j * 128:j * 128 + tw],
                            idenf_sb[:])
                        tb = tb_pool.tile([128, 128], F32, name="tb", tag="tb")
                        nc.scalar.copy(tb[0:tw, :], ptp[0:tw, :])
                        nc.sync.dma_start(
                            out_d.ap()[j * 128:j * 128 + tw,
                                       l * O:(l + 1) * O],
                            tb[0:tw, :])

    _split_excess_waits(nc)
    return nc


def host_prep(x0c, k0, k1, k2):
    """Per-core input prep. x0c: (bcl, M, D) float32."""
    bcl = x0c.shape[0]
    x0m = np.ascontiguousarray(
        x0c.transpose(1, 0, 2).reshape(M, bcl * D), dtype=np.float32)
    x0mb = x0m.astype(ml_dtypes.bfloat16)
    x0p2 = np.zeros((128, bcl * D), ml_dtypes.bfloat16)
    x0p2[0:M] = x0mb
    x0p2[64:64 + M] = x0mb
    x0t = np.concatenate(
        [x0c.transpose(0, 2, 1).reshape(bcl * D, M),
         np.ones((bcl * D, 1), np.float32)], axis=1)
    x0t = np.ascontiguousarray(x0t, dtype=np.float32)
    k0p = np.ascontiguousarray(
        k0.transpose(1, 2, 0).reshape(M, M * O), dtype=np.float32)
    k0pb = k0p.astype(ml_dtypes.bfloat16)
    npair = GROUPS // 2
    k0p2 = np.zeros((128, M * O // 2), ml_dtypes.bfloat16)
    for p in range(npair):
        ps = slice(p * MPG * O, (p + 1) * MPG * O)
        k0p2[0:M, ps] = k0pb[:, (2 * p) * MPG * O:(2 * p + 1) * MPG * O]
        k0p2[64:64 + M, ps] = k0pb[:, (2 * p + 1) * MPG * O:
                                   (2 * p + 2) * MPG * O]
    k1p = np.ascontiguousarray(
        k1.transpose(1, 2, 0).reshape(O, M * O)).astype(ml_dtypes.bfloat16)
    k2p = np.ascontiguousarray(
        k2.transpose(1, 2, 0).reshape(O, M * O)).astype(ml_dtypes.bfloat16)
    e8 = (np.arange(128)[:, None] // D == np.arange(8)[None, :])
    e8b = e8.astype(ml_dtypes.bfloat16)
    e41 = np.ascontiguousarray(
        np.tile(e8b[:, None, :], (1, M1, 1)).reshape(128, M1 * 8))
    idenb = np.eye(128, dtype=ml_dtypes.bfloat16)
    idenf = np.eye(128, dtype=np.float32)
    return {"x0p2": x0p2, "x0t": x0t, "k0p2": k0p2, "k1p": k1p,
            "k2p": k2p, "e41": e41, "idenb": idenb, "idenf": idenf}


_nc_cache = {}


def _get_nc(n_chunks):
    if n_chunks not in _nc_cache:
        _nc_cache[n_chunks] = build(n_chunks)
    return _nc_cache[n_chunks]


def kernel(x0, k0, k1, k2):
    from concourse.bass_utils import run_bass_kernel_spmd
    x0 = np.asarray(x0, dtype=np.float32)
    k0 = np.asarray(k0, dtype=np.float32)
    k1 = np.asarray(k1, dtype=np.float32)
    k2 = np.asarray(k2, dtype=np.float32)
    n_chunks = (BC * D) // 128
    nc = _get_nc(n_chunks)
    in_maps = [host_prep(x0[c * BC:(c + 1) * BC], k0, k1, k2)
               for c in range(N_CORES)]
    res = run_bass_kernel_spmd(nc, in_maps, core_ids=list(range(N_CORES)))
    out = np.concatenate([r["out"] for r in res.results], axis=0)
    return out.astype(np.float32)
